# revision 11
# baseline (speedup 1.0000x reference)
"""StyleGAN2 modulated+demodulated 3x3 conv (B=8, H=W=C=F=128) on 8 trn2 cores.

Strategy: batch-parallel (1 sample per core). The grouped conv is rewritten as
a plain conv with per-sample folded weights:
    s[c]    = style[c] + 1
    wmod    = kernel * s[c]                  (modulate)
    d2[f]   = sum_{kh,kw,c} wmod^2           (demod denominator, fp32)
    wfin    = wmod / sqrt(d2 + 1e-8)         (folded, cast to bf16)
    y       = conv2d(x, wfin, SAME)          (bf16 matmuls, fp32 PSUM accum)

Layout: x is transposed on-chip to channel-major [c, (w-major padded flat)]
using TensorE transposes of [h, c] slices (bf16, cheap). Output rows
(fixed w, all h) come out of PSUM directly in [h, f] order -> NHWC stores.
"""

import os
import numpy as np
from contextlib import ExitStack

B, H, W, C, F, KS = 8, 128, 128, 128, 128, 3
P = 128
RS = W + 2  # padded row stride (row = fixed w, cols = h with 1-halo)
NPAD = (W + 2) * RS
CHUNK = 16  # w's per input DMA chunk
OBLK = 16   # output w's per store DMA

_NC = None


def _build():
    import concourse.bass as bass
    import concourse.tile as tile
    from concourse import bacc, masks, mybir

    f32 = mybir.dt.float32
    bf16 = mybir.dt.bfloat16
    AF = mybir.ActivationFunctionType

    nc = bacc.Bacc("TRN2", target_bir_lowering=False, debug=False, num_devices=B)

    x_d = nc.declare_dram_parameter("x", [H, W, C], f32, isOutput=False)
    s_d = nc.declare_dram_parameter("style", [C, 1], f32, isOutput=False)
    k_d = nc.declare_dram_parameter("kern", [KS, KS, C, F], f32, isOutput=False)
    y_d = nc.declare_dram_parameter("y", [H, W, F], f32, isOutput=True)

    with tile.TileContext(nc) as tc, ExitStack() as ctx:
        const = ctx.enter_context(tc.tile_pool(name="const", bufs=1))
        xbp = ctx.enter_context(tc.tile_pool(name="xb", bufs=1))
        xcmp = ctx.enter_context(tc.tile_pool(name="xcm", bufs=1))
        ost = ctx.enter_context(tc.tile_pool(name="ost", bufs=2))

        ident = const.tile([P, P], bf16)
        masks.make_identity(nc, ident[:])

        # ---- weight prep (all fp32 until the final bf16 cast) ----
        w_f32 = const.tile([C, KS * KS * F], f32)   # [c, (a b f)]
        nc.sync.dma_start(w_f32[:], k_d[:].rearrange("a b c f -> c a b f"))
        s_raw = const.tile([C, 1], f32)
        nc.sync.dma_start(s_raw[:], s_d[:])

        s_sb = const.tile([C, 1], f32)
        nc.scalar.activation(s_sb[:], s_raw[:], AF.Copy, bias=1.0)  # s = style+1

        w_mod = const.tile([C, KS * KS * F], f32)
        nc.vector.tensor_scalar_mul(w_mod[:], w_f32[:], s_sb[:, 0:1])

        sq = const.tile([C, KS * KS * F], f32)
        nc.vector.tensor_mul(sq[:], w_mod[:], w_mod[:])
        ksum = const.tile([C, F], f32)
        nc.vector.tensor_add(ksum[:], sq[:, 0:F], sq[:, F : 2 * F])
        for t in range(2, KS * KS):
            nc.vector.tensor_add(ksum[:], ksum[:], sq[:, t * F : (t + 1) * F])

        ones_col = const.tile([C, 1], f32)
        nc.gpsimd.memset(ones_col[:], 1.0)
        ones_row = const.tile([1, C], f32)
        nc.gpsimd.memset(ones_row[:], 1.0)
        eps = const.tile([1, 1], f32)
        nc.gpsimd.memset(eps[:], 1e-8)

        with tc.tile_pool(name="psum_misc", bufs=2, space="PSUM") as pmisc:
            d2_ps = pmisc.tile([1, F], f32)
            nc.tensor.matmul(d2_ps[:], ones_col[:], ksum[:])  # d2[f] = sum_c
            sd = const.tile([1, F], f32)
            nc.scalar.activation(sd[:], d2_ps[:], AF.Sqrt, bias=eps[:, 0:1])
            invd = const.tile([1, F], f32)
            nc.vector.reciprocal(invd[:], sd[:])

            bc_ps = pmisc.tile([C, F], f32)
            nc.tensor.matmul(bc_ps[:], ones_row[:], invd[:])  # bcast over c
            bc_sb = const.tile([C, F], f32)
            nc.scalar.copy(bc_sb[:], bc_ps[:])

        w_bf = const.tile([C, KS * KS * F], bf16)
        for t in range(KS * KS):
            nc.vector.tensor_mul(
                w_bf[:, t * F : (t + 1) * F], w_mod[:, t * F : (t + 1) * F], bc_sb[:]
            )

        # ---- x load (cast to bf16 in DMA) ----
        x_bf = xbp.tile([P, W * C], bf16)  # [h, (w c)]
        for k in range(W // CHUNK):
            nc.gpsimd.dma_start(
                x_bf[:, k * CHUNK * C : (k + 1) * CHUNK * C],
                x_d[:, k * CHUNK : (k + 1) * CHUNK, :],
            )

        # ---- channel-major padded buffer ----
        x_cm = xcmp.tile([C, NPAD], bf16)  # [c, (w+1)*RS + (h+1)]
        nc.gpsimd.memset(x_cm[:, 0:RS], 0.0)                 # w = -1 row
        nc.gpsimd.memset(x_cm[:, (W + 1) * RS : NPAD], 0.0)  # w = W row
        x_cm_3d = x_cm[:].rearrange("c (r j) -> c r j", j=RS)
        nc.gpsimd.memset(x_cm_3d[:, 1 : W + 1, 0], 0.0)      # h = -1 col
        nc.gpsimd.memset(x_cm_3d[:, 1 : W + 1, RS - 1], 0.0) # h = H col

        pst = ctx.enter_context(tc.tile_pool(name="psum_t", bufs=3, space="PSUM"))
        pso = ctx.enter_context(tc.tile_pool(name="psum_o", bufs=4, space="PSUM"))

        def emit_transpose(wp):
            # x_bf[:, wp] is [h, c]; transpose -> [c, h] into padded row wp+1
            tp = pst.tile([C, P], bf16)
            nc.tensor.transpose(tp[:], x_bf[:, wp * C : (wp + 1) * C], ident[:])
            dst = x_cm[:, (wp + 1) * RS + 1 : (wp + 1) * RS + 1 + H]
            if wp % 2 == 0:
                nc.vector.tensor_copy(dst, tp[:])
            else:
                nc.scalar.copy(dst, tp[:])

        emit_transpose(0)
        emit_transpose(1)

        pq = {}  # live PSUM accum tiles by output w
        cur_stage = None

        for rpad in range(W + 2):
            if rpad + 2 < W:
                emit_transpose(rpad + 2)
            if rpad < W:
                pq[rpad] = pso.tile([P, F], f32, name="pq", tag="pq")
            for a in range(KS):
                lhsT = x_cm[:, rpad * RS + a : rpad * RS + a + P]
                for b in range(KS):
                    q = rpad - b
                    if q < 0 or q >= W:
                        continue
                    nc.tensor.matmul(
                        pq[q][:],
                        lhsT,
                        w_bf[:, (a * KS + b) * F : (a * KS + b + 1) * F],
                        start=(a == 0 and b == 0),
                        stop=(a == KS - 1 and b == KS - 1),
                    )
            qd = rpad - 2
            if 0 <= qd < W:
                if qd % OBLK == 0:
                    cur_stage = ost.tile([P, OBLK * F], f32, name="stage", tag="stage")
                dst = cur_stage[:, (qd % OBLK) * F : (qd % OBLK + 1) * F]
                if qd % 2 == 0:
                    nc.scalar.copy(dst, pq[qd][:])
                else:
                    nc.vector.tensor_copy(dst, pq[qd][:])
                del pq[qd]
                if qd % OBLK == OBLK - 1:
                    q0 = qd - (OBLK - 1)
                    nc.sync.dma_start(y_d[:, q0 : q0 + OBLK, :], cur_stage[:])

    nc.finalize()
    return nc


def _get_nc():
    global _NC
    if _NC is None:
        _NC = _build()
    return _NC


def _in_maps(x, style, kern):
    x = np.ascontiguousarray(np.asarray(x), dtype=np.float32)
    style = np.ascontiguousarray(np.asarray(style), dtype=np.float32)
    kern = np.ascontiguousarray(np.asarray(kern), dtype=np.float32)
    return [
        {"x": x[i], "style": style[i].reshape(C, 1), "kern": kern} for i in range(B)
    ]


def _ensure_ntff_hook():
    """The agent image's antenv lacks axon_hooks; recreate the ctypes NTFF
    profile hook (same recipe as trn_agent_boot.trn_boot) so trace=True works."""
    import sys, types, contextlib, ctypes

    try:
        import antenv.axon_hooks  # noqa: F401
        return
    except ImportError:
        pass

    _hook = None
    try:
        lib = ctypes.CDLL("/opt/axon/libaxon_pjrt.so")
        if hasattr(lib, "axon_start_nrt_profile"):
            lib.axon_start_nrt_profile.argtypes = [
                ctypes.POINTER(ctypes.c_int64),
                ctypes.c_size_t,
            ]
            lib.axon_start_nrt_profile.restype = ctypes.c_int64
            lib.axon_stop_nrt_profile.argtypes = [ctypes.c_char_p]
            lib.axon_stop_nrt_profile.restype = ctypes.c_int64

            @contextlib.contextmanager
            def _hook_cm(output_dir, device_ids):
                import jax

                jax.devices()
                if device_ids:
                    ids = (ctypes.c_int64 * len(device_ids))(*device_ids)
                    rc = lib.axon_start_nrt_profile(ids, len(device_ids))
                else:
                    rc = lib.axon_start_nrt_profile(None, 0)
                if rc != 0:
                    raise RuntimeError(f"axon_start_nrt_profile rc={rc}")
                try:
                    yield
                finally:
                    n = lib.axon_stop_nrt_profile(str(output_dir).encode())
                    print(f"ntff profile: {n} file(s) -> {output_dir}")

            _hook = _hook_cm
    except OSError:
        pass

    mod = types.ModuleType("antenv.axon_hooks")
    mod.get_axon_ntff_profile_hook = lambda: _hook
    mod.set_axon_ntff_profile_hook = lambda h: None
    sys.modules["antenv.axon_hooks"] = mod


def run(x, style, kern, trace=False, trace_cores=None):
    from concourse.bass_utils import run_bass_kernel_spmd

    if trace:
        _ensure_ntff_hook()
    nc = _get_nc()
    res = run_bass_kernel_spmd(
        nc,
        _in_maps(x, style, kern),
        list(range(B)),
        trace=trace,
        trace_cores=trace_cores,
    )
    y = np.stack([res.results[i]["y"] for i in range(B)]).astype(np.float32)
    return y, res


def kernel(x, style, kernel):
    y, _ = run(x, style, kernel, trace=False)
    return y
